# revision 53
# baseline (speedup 1.0000x reference)
"""MoE SwiGLU experts kernel for Trainium2, 8 NeuronCores.

Strategy: expert-pair parallel with F-split.
  - Tokens are sorted by expert on the host (argsort of expert_idx).
  - Cores 2i and 2i+1 jointly own experts (2i, 2i+1): both cores get the
    same token set (experts 2i & 2i+1, padded to `cap` tokens each), but
    core 2i uses columns [0:2048] of w_gate/w_up (rows of w_down) and
    core 2i+1 uses columns [2048:4096].  Each core runs the full SwiGLU
    over its F-half; the host sums the two partial down-projections.
  - On-chip layout is feature-major (partitions = feature dim, free dim =
    tokens), so w_gate/w_up/w_down natural layouts serve directly as the
    stationary lhsT operand: out = lhsT.T @ rhs.  Zero on-chip transposes.
"""

import math

import numpy as np
import orjson

import concourse.bass as bass
import concourse.mybir as mybir
import concourse.tile as tile
from concourse import bass2jax
from concourse.bass_utils import run_bass_kernel_spmd

FP32 = mybir.dt.float32
FP16 = mybir.dt.float16
BF16 = mybir.dt.bfloat16
FP32R = mybir.dt.float32r

# compute/storage dtypes.  DT1: phase-1 inputs (x, w_gate, w_up); DT2:
# phase-2 operands (h, w_down).  fp32r measured fastest on HW (354us/rep vs
# ~363us for fp16/bf16 all-16bit, old structure) and most accurate; 16-bit
# gains nothing at the PE (1 row/cycle either way) but halves the DMA
# stream, which only matters for the cold-start transient.
DT1 = FP32R
DT2 = FP32R
NP_DT1 = np.float32
NP_DT2 = np.float32
WP_BUFS = 3
X_WHOLE = False

D = 1024        # model dim
F = 4096        # ffn dim
F2 = F // 2     # per-core F half
E = 8           # experts
KD = D // 128   # 8  k-tiles over D
KF = F2 // 128  # 16 f-tiles over F half


# ---------------------------------------------------------------------------
# BIR legalizer: this container's walrus accepts at most ONE sync-wait per
# instruction.  Tile emits instructions with several waits; hoist the excess
# onto preceding EventSemaphore carrier instructions on the same engine
# (engines execute their stream in order, so waiting earlier is equivalent).
# ---------------------------------------------------------------------------

def _legalize_bir_waits(bir_bytes: bytes) -> bytes:
    bir = orjson.loads(bir_bytes)
    n_fix = 0
    for f in bir.get("functions", []):
        for b in f.get("blocks", []):
            out = []
            for inst in b.get("instructions", []):
                si = inst.get("sync_info")
                waits = (si or {}).get("on_wait") or []
                if len(waits) > 1:
                    keep = waits[-1:]
                    excess = waits[:-1]
                    for w in excess:
                        n_fix += 1
                        out.append({
                            "debug": inst.get("debug", 0),
                            "engine": inst["engine"],
                            "ins": [], "outs": [],
                            "name": f"wfix-{n_fix}-{inst['name']}",
                            "opcode": "EventSemaphore",
                            "sync_info": {"on_update": [], "on_wait": [w]},
                        })
                    si["on_wait"] = keep
                out.append(inst)
            b["instructions"] = out
    return orjson.dumps(bir)


_orig_decompress = bass2jax._decompress_ant_bir


def _patched_decompress(v):
    return _legalize_bir_waits(_orig_decompress(v))


bass2jax._decompress_ant_bir = _patched_decompress


# ---------------------------------------------------------------------------
# Device program (SPMD, identical across the 8 cores; per-core inputs differ)
# ---------------------------------------------------------------------------

def _chunks(cap):
    """Split the token free-dim into matmul chunks: <=512 wide (one fp32 PSUM
    bank), multiples of 16, >=256 each (float32r matmuls run at full rate
    only when the moving free-dim is >=256).  The first chunk is clamped to
    256 so the first matmul chain at kernel start waits on the fewest DMA
    bytes."""
    n = max(1, math.ceil(cap / 512))
    base = (cap // n) // 16 * 16
    rem = (cap - n * base) // 16
    widths = [base + 16] * rem + [base] * (n - rem)
    if len(widths) > 1 and widths[0] > 256:
        spare = widths[0] - 256
        widths[0] = 256
        for i in range(1, len(widths)):
            add = min(spare, 512 - widths[i])
            widths[i] += add
            spare -= add
        widths[0] += spare  # not redistributable (all others at 512)
    out = []
    c0 = 0
    for w in widths:
        out.append((c0, w))
        c0 += w
    return out


def _tail_chunks(chunks):
    """Chunk order for the very last d-tile: narrowest chunk last so the
    final flush (chain + copy + DMA) after the last matmul is minimal; if
    the narrowest is still >=512, split 256 off its end."""
    rest = sorted(chunks, key=lambda cw: cw[1])
    narrow = rest.pop(0)
    out = sorted(rest) + [narrow]
    c0, w = out[-1]
    if w >= 512:
        out[-1] = (c0, w - 256)
        out.append((c0 + w - 256, 256))
    return out


def build_program(caps, reps: int = 1, dt1=None, dt2=None) -> bass.Bass:
    """caps = (cap0, cap1): token capacity of unit 0 (the 4 largest experts,
    runs cold at kernel start) and unit 1 (the 4 smallest; 2x512 chunks when
    balanced routing gives cap1 <= 1024)."""
    cap0, cap1 = caps
    dt1 = dt1 or DT1
    dt2 = dt2 or DT2
    nc = bass.Bass()

    # unit a: the larger expert of the pair; unit b: the smaller.
    # x layout [128, KD, cap]: partition-major so the whole activation set
    # loads as ONE contiguous DMA; matmuls slice x[:, k, c0:c0+w].
    xa = nc.declare_dram_parameter("xa", [128, KD, cap0], dt1, isOutput=False)
    xb = nc.declare_dram_parameter("xb", [128, KD, cap1], dt1, isOutput=False)
    wga = nc.declare_dram_parameter("wga", [KF, 128, KD, 128], dt1, isOutput=False)
    wua = nc.declare_dram_parameter("wua", [KF, 128, KD, 128], dt1, isOutput=False)
    wda = nc.declare_dram_parameter("wda", [KD, 128, KF, 128], dt2, isOutput=False)
    wgb = nc.declare_dram_parameter("wgb", [KF, 128, KD, 128], dt1, isOutput=False)
    wub = nc.declare_dram_parameter("wub", [KF, 128, KD, 128], dt1, isOutput=False)
    wdb = nc.declare_dram_parameter("wdb", [KD, 128, KF, 128], dt2, isOutput=False)
    ya = nc.declare_dram_parameter("ya", [KD, 128, cap0], FP32, isOutput=True)
    yb = nc.declare_dram_parameter("yb", [KD, 128, cap1], FP32, isOutput=True)

    units = [(xa, wga, wua, wda, ya, cap0, _chunks(cap0)),
             (xb, wgb, wub, wdb, yb, cap1, _chunks(cap1))] * reps
    last_unit = len(units) - 1

    with tile.TileContext(nc) as tc:
        with (
            tc.tile_pool(name="xp", bufs=1) as xp,
            tc.tile_pool(name="hp", bufs=1) as hp,
            tc.tile_pool(name="wp", bufs=WP_BUFS) as wp,
            tc.tile_pool(name="sp", bufs=2) as sp,
            tc.tile_pool(name="yp", bufs=2) as yp,
            tc.tile_pool(name="ps", bufs=2, space=bass.MemorySpace.PSUM) as ps,
        ):
            xt_next = None
            for u_i, (x_d, wg_d, wu_d, wd_d, y_d, cap, chunks) in \
                    enumerate(units):
                # token activations, feature-major: xt[:, k, :] = X^T[k-tile]
                if xt_next is None:
                    xt = xp.tile([128, KD, cap], dt1, tag="x")
                else:
                    xt = xt_next  # prefetched during the previous phase 2
                xt_next = None

                # H^T for this unit: [f (128 part), f-tile (16), tokens]
                h = hp.tile([128, KF, cap], dt2, tag="h")

                # ---- phase 1: G/U = X @ Wg, X @ Wu ; H = silu(G) * U ----
                for fj in range(KF):
                    wgt = wp.tile([128, KD, 128], dt1, tag="wg")
                    wut = wp.tile([128, KD, 128], dt1, tag="wu")
                    if fj == 0 and u_i == 0:
                        # cold-start order: interleave the first weight tile
                        # (in halves) with x chunk 0 per k-tile so the first
                        # matmul chain is gated on the fewest possible bytes
                        c0, w = chunks[0]
                        nc.sync.dma_start(
                            wgt[:, :KD // 2], wg_d[fj, :, :KD // 2])
                        for k in range(KD // 2):
                            nc.sync.dma_start(
                                xt[:, k, c0:c0 + w], x_d[:, k, c0:c0 + w])
                        nc.sync.dma_start(
                            wgt[:, KD // 2:], wg_d[fj, :, KD // 2:])
                        for k in range(KD // 2, KD):
                            nc.sync.dma_start(
                                xt[:, k, c0:c0 + w], x_d[:, k, c0:c0 + w])
                        nc.sync.dma_start(wut[:], wu_d[fj])
                        if len(chunks) > 1:
                            r0 = chunks[1][0]  # remainder of x, per k-tile
                            for k in range(KD):
                                nc.sync.dma_start(
                                    xt[:, k, r0:cap], x_d[:, k, r0:cap])
                    else:
                        nc.sync.dma_start(wgt[:], wg_d[fj])
                        nc.sync.dma_start(wut[:], wu_d[fj])
                    for (c0, w) in chunks:
                        g_ps = ps.tile([128, w], FP32, tag="g")
                        for k in range(KD):
                            nc.tensor.matmul(
                                g_ps[:, :w],
                                wgt[:, k, :], xt[:, k, c0:c0 + w],
                                start=(k == 0), stop=(k == KD - 1),
                            )
                        u_ps = ps.tile([128, w], FP32, tag="u")
                        for k in range(KD):
                            nc.tensor.matmul(
                                u_ps[:, :w],
                                wut[:, k, :], xt[:, k, c0:c0 + w],
                                start=(k == 0), stop=(k == KD - 1),
                            )
                        sg = sp.tile([128, w], FP32, tag="sg")
                        nc.scalar.activation(
                            sg[:, :w], g_ps[:, :w],
                            mybir.ActivationFunctionType.Silu,
                        )
                        nc.vector.tensor_mul(
                            h[:, fj, c0:c0 + w], sg[:, :w], u_ps[:, :w]
                        )

                # ---- phase 2: Y^T partial = Wd^T @ H^T (accum over f) ----
                for d in range(KD):
                    wdt = wp.tile([128, KF, 128], dt2, tag="wd")
                    nc.sync.dma_start(wdt[:], wd_d[d])
                    if u_i < last_unit and d < 4:
                        # prefetch the next unit's x in quarters, interleaved
                        # with the wd stream so no single transfer
                        # head-of-line-blocks the following phase-1 weights
                        xn_d, cap_n = units[u_i + 1][0], units[u_i + 1][5]
                        if d == 0:
                            xt_next = xp.tile([128, KD, cap_n], dt1, tag="x")
                        k0, k1 = 2 * d, 2 * d + 2
                        nc.sync.dma_start(
                            xt_next[:, k0:k1], xn_d[:, k0:k1])
                    y_sb = yp.tile([128, cap], FP32, tag="y")
                    tail = u_i == last_unit and d == KD - 1
                    d_chunks = _tail_chunks(chunks) if tail else chunks
                    for (c0, w) in d_chunks:
                        y_ps = ps.tile([128, w], FP32, tag="yp")
                        for fj in range(KF):
                            nc.tensor.matmul(
                                y_ps[:, :w],
                                wdt[:, fj, :], h[:, fj, c0:c0 + w],
                                start=(fj == 0), stop=(fj == KF - 1),
                            )
                        nc.vector.tensor_copy(y_sb[:, c0:c0 + w], y_ps[:, :w])
                        if tail:
                            # per-chunk drain so the final flush after the
                            # last matmul is a single narrow chunk
                            nc.scalar.dma_start(
                                y_d[d][:, c0:c0 + w], y_sb[:, c0:c0 + w])
                    if not tail:
                        # one DMA per d-tile, on the Activation queue:
                        # outputs must not head-of-line-block input prefetch
                        # on the SP queue
                        nc.scalar.dma_start(y_d[d], y_sb[:])

    return nc


# ---------------------------------------------------------------------------
# Host-side sharding / unsharding
# ---------------------------------------------------------------------------

def _prep_w_gate_like(w_slice):
    """[D, F2] -> [KF, 128, KD, 128] so each f-tile is one contiguous DMA."""
    arr = w_slice.reshape(KD, 128, KF, 128)       # [k, p, fj, f]
    return np.ascontiguousarray(arr.transpose(2, 1, 0, 3)).astype(NP_DT1)


def _prep_w_down(w_slice):
    """[F2, D] -> [KD, 128, KF, 128] so each d-tile is one contiguous DMA."""
    arr = w_slice.reshape(KF, 128, KD, 128)       # [fj, p, d, dc]
    return np.ascontiguousarray(arr.transpose(2, 1, 0, 3)).astype(NP_DT2)


def _prep_x(x_tok, cap):
    """[count, D] tokens -> [128, KD, cap] partition-major padded."""
    count = x_tok.shape[0]
    xt = np.zeros((D, cap), dtype=NP_DT1)
    xt[:, :count] = x_tok.T.astype(NP_DT1)
    return np.ascontiguousarray(xt.reshape(KD, 128, cap).transpose(1, 0, 2))


_prog_cache = {}


def get_program(caps) -> bass.Bass:
    if caps not in _prog_cache:
        _prog_cache[caps] = build_program(caps)
    return _prog_cache[caps]


def _pad16(n):
    return max(128, int(math.ceil(max(int(n), 1) / 16)) * 16)


def prepare_in_maps(np_inputs):
    """Host-side sharding.  Returns (in_maps, caps, meta) where meta carries
    what's needed to unshard.

    Experts are sorted by token count: the 4 largest go to unit a (cap0),
    the 4 smallest to unit b (cap1).  This minimizes cap0+cap1 (the per-core
    PE time is proportional to it) and, for balanced routing, gives unit b
    cap1 <= 1024 = 2x512-wide PSUM chunks."""
    x = np.asarray(np_inputs["x"])
    B, S, _ = x.shape
    xf = np.ascontiguousarray(x.reshape(-1, D).astype(np.float32, copy=False))
    idx = np.asarray(np_inputs["expert_idx"]).reshape(-1)
    w_gate = np.asarray(np_inputs["w_gate"], dtype=np.float32)
    w_up = np.asarray(np_inputs["w_up"], dtype=np.float32)
    w_down = np.asarray(np_inputs["w_down"], dtype=np.float32)

    order = np.argsort(idx, kind="stable")
    counts = np.bincount(idx, minlength=E).astype(np.int64)
    starts = np.zeros(E + 1, dtype=np.int64)
    np.cumsum(counts, out=starts[1:])

    by_count = np.argsort(-counts, kind="stable")
    big, small = by_count[:E // 2], by_count[E // 2:]
    cap0 = _pad16(counts[big].max())
    cap1 = _pad16(counts[small].max())
    caps = (cap0, cap1)

    tok_of = [order[starts[e]:starts[e + 1]] for e in range(E)]

    in_maps = []
    for pair in range(4):
        ea, eb = int(big[pair]), int(small[pair])
        xa = _prep_x(xf[tok_of[ea]], cap0)
        xb = _prep_x(xf[tok_of[eb]], cap1)
        for hhalf in range(2):
            sl = slice(hhalf * F2, (hhalf + 1) * F2)
            in_maps.append({
                "xa": xa,
                "xb": xb,
                "wga": _prep_w_gate_like(w_gate[ea][:, sl]),
                "wua": _prep_w_gate_like(w_up[ea][:, sl]),
                "wda": _prep_w_down(w_down[ea][sl, :]),
                "wgb": _prep_w_gate_like(w_gate[eb][:, sl]),
                "wub": _prep_w_gate_like(w_up[eb][:, sl]),
                "wdb": _prep_w_down(w_down[eb][sl, :]),
            })
    meta = (tok_of, counts, (B, S), x.dtype, big, small)
    return in_maps, caps, meta


def unshard(results, caps, meta):
    tok_of, counts, (B, S), out_dtype, big, small = meta
    cap0, cap1 = caps
    out = np.zeros((B * S, D), dtype=np.float32)
    for pair in range(4):
        r0 = results[2 * pair]
        r1 = results[2 * pair + 1]
        for e, key, cap in ((int(big[pair]), "ya", cap0),
                            (int(small[pair]), "yb", cap1)):
            yt = (np.asarray(r0[key], dtype=np.float32)
                  + np.asarray(r1[key], dtype=np.float32)).reshape(D, cap)
            out[tok_of[e]] = yt[:, :counts[e]].T
    return out.reshape(B, S, D).astype(out_dtype, copy=False)


def kernel(x, expert_idx, w_gate, w_up, w_down):
    np_inputs = {"x": x, "expert_idx": expert_idx, "w_gate": w_gate,
                 "w_up": w_up, "w_down": w_down}
    in_maps, caps, meta = prepare_in_maps(np_inputs)
    nc = get_program(caps)
    try:
        res = run_bass_kernel_spmd(nc, in_maps, list(range(8)))
    except Exception:
        # transient device errors have been observed on this fabric; one retry
        res = run_bass_kernel_spmd(nc, in_maps, list(range(8)))
    return unshard(res.results, caps, meta)



# revision 60
# speedup vs baseline: 1.1717x; 1.1717x over previous
"""MoE SwiGLU experts kernel for Trainium2, 8 NeuronCores.

Strategy: expert-pair parallel with F-split.
  - Tokens are sorted by expert on the host (argsort of expert_idx).
  - Cores 2i and 2i+1 jointly own experts (2i, 2i+1): both cores get the
    same token set (experts 2i & 2i+1, padded to `cap` tokens each), but
    core 2i uses columns [0:2048] of w_gate/w_up (rows of w_down) and
    core 2i+1 uses columns [2048:4096].  Each core runs the full SwiGLU
    over its F-half; the host sums the two partial down-projections.
  - On-chip layout is feature-major (partitions = feature dim, free dim =
    tokens), so w_gate/w_up/w_down natural layouts serve directly as the
    stationary lhsT operand: out = lhsT.T @ rhs.  Zero on-chip transposes.
"""

import math

import numpy as np
import orjson

import concourse.bass as bass
import concourse.mybir as mybir
import concourse.tile as tile
from concourse import bass2jax
from concourse.bass_utils import run_bass_kernel_spmd

FP32 = mybir.dt.float32
FP16 = mybir.dt.float16
BF16 = mybir.dt.bfloat16
FP32R = mybir.dt.float32r

# compute/storage dtypes.  DT1: phase-1 inputs (x, w_gate, w_up); DT2:
# phase-2 operands (h, w_down).  fp32r measured fastest on HW (354us/rep vs
# ~363us for fp16/bf16 all-16bit, old structure) and most accurate; 16-bit
# gains nothing at the PE (1 row/cycle either way) but halves the DMA
# stream, which only matters for the cold-start transient.
DT1 = FP32R
DT2 = FP32R
NP_DT1 = np.float32
NP_DT2 = np.float32
WP_BUFS = 3
X_WHOLE = False

D = 1024        # model dim
F = 4096        # ffn dim
F2 = F // 2     # per-core F half
E = 8           # experts
KD = D // 128   # 8  k-tiles over D
KF = F2 // 128  # 16 f-tiles over F half


# ---------------------------------------------------------------------------
# BIR legalizer: this container's walrus accepts at most ONE sync-wait per
# instruction.  Tile emits instructions with several waits; hoist the excess
# onto preceding EventSemaphore carrier instructions on the same engine
# (engines execute their stream in order, so waiting earlier is equivalent).
# ---------------------------------------------------------------------------

def _legalize_bir_waits(bir_bytes: bytes) -> bytes:
    bir = orjson.loads(bir_bytes)
    n_fix = 0
    for f in bir.get("functions", []):
        for b in f.get("blocks", []):
            out = []
            for inst in b.get("instructions", []):
                si = inst.get("sync_info")
                waits = (si or {}).get("on_wait") or []
                if len(waits) > 1:
                    keep = waits[-1:]
                    excess = waits[:-1]
                    for w in excess:
                        n_fix += 1
                        out.append({
                            "debug": inst.get("debug", 0),
                            "engine": inst["engine"],
                            "ins": [], "outs": [],
                            "name": f"wfix-{n_fix}-{inst['name']}",
                            "opcode": "EventSemaphore",
                            "sync_info": {"on_update": [], "on_wait": [w]},
                        })
                    si["on_wait"] = keep
                out.append(inst)
            b["instructions"] = out
    return orjson.dumps(bir)


_orig_decompress = bass2jax._decompress_ant_bir


def _patched_decompress(v):
    return _legalize_bir_waits(_orig_decompress(v))


bass2jax._decompress_ant_bir = _patched_decompress


# ---------------------------------------------------------------------------
# Device program (SPMD, identical across the 8 cores; per-core inputs differ)
# ---------------------------------------------------------------------------

def _chunks(cap):
    """Split the token free-dim into matmul chunks: <=512 wide (one fp32 PSUM
    bank), multiples of 16, >=256 each (float32r matmuls run at full rate
    only when the moving free-dim is >=256).  The first chunk is clamped to
    256 so the first matmul chain at kernel start waits on the fewest DMA
    bytes."""
    n = max(1, math.ceil(cap / 512))
    base = (cap // n) // 16 * 16
    rem = (cap - n * base) // 16
    widths = [base + 16] * rem + [base] * (n - rem)
    if len(widths) > 1 and widths[0] > 256:
        spare = widths[0] - 256
        widths[0] = 256
        for i in range(1, len(widths)):
            add = min(spare, 512 - widths[i])
            widths[i] += add
            spare -= add
        widths[0] += spare  # not redistributable (all others at 512)
    out = []
    c0 = 0
    for w in widths:
        out.append((c0, w))
        c0 += w
    return out


def _tail_chunks(chunks):
    """Chunk order for the very last d-tile: narrowest chunk last so the
    final flush (chain + copy + DMA) after the last matmul is minimal; if
    the narrowest is still >=512, split 256 off its end."""
    rest = sorted(chunks, key=lambda cw: cw[1])
    narrow = rest.pop(0)
    out = sorted(rest) + [narrow]
    c0, w = out[-1]
    if w >= 512:
        out[-1] = (c0, w - 256)
        out.append((c0 + w - 256, 256))
    return out


def build_program(caps, reps: int = 1, dt1=None, dt2=None) -> bass.Bass:
    """caps = (cap0, cap1): token capacity of unit 0 (the 4 largest experts,
    runs cold at kernel start) and unit 1 (the 4 smallest; 2x512 chunks when
    balanced routing gives cap1 <= 1024)."""
    cap0, cap1 = caps
    dt1 = dt1 or DT1
    dt2 = dt2 or DT2
    nc = bass.Bass()

    # unit a: the larger expert of the pair; unit b: the smaller.
    # x layout [128, KD, cap]: partition-major so the whole activation set
    # loads as ONE contiguous DMA; matmuls slice x[:, k, c0:c0+w].
    # xa ships as fp16 (halves the cold-start DMA transient, which is
    # x-bandwidth-bound) and is converted to dt1 on-chip by the Activation
    # engine; xb's transfer is fully hidden in unit-a's phase-2 slack, so it
    # ships in dt1 directly.
    xa = nc.declare_dram_parameter("xa", [128, KD, cap0], FP16, isOutput=False)
    xb = nc.declare_dram_parameter("xb", [128, KD, cap1], dt1, isOutput=False)
    wga = nc.declare_dram_parameter("wga", [KF, 128, KD, 128], dt1, isOutput=False)
    wua = nc.declare_dram_parameter("wua", [KF, 128, KD, 128], dt1, isOutput=False)
    wda = nc.declare_dram_parameter("wda", [KD, 128, KF, 128], dt2, isOutput=False)
    wgb = nc.declare_dram_parameter("wgb", [KF, 128, KD, 128], dt1, isOutput=False)
    wub = nc.declare_dram_parameter("wub", [KF, 128, KD, 128], dt1, isOutput=False)
    wdb = nc.declare_dram_parameter("wdb", [KD, 128, KF, 128], dt2, isOutput=False)
    ya = nc.declare_dram_parameter("ya", [KD, 128, cap0], FP32, isOutput=True)
    yb = nc.declare_dram_parameter("yb", [KD, 128, cap1], FP32, isOutput=True)

    units = [(xa, wga, wua, wda, ya, cap0, _chunks(cap0)),
             (xb, wgb, wub, wdb, yb, cap1, _chunks(cap1))] * reps
    last_unit = len(units) - 1

    with tile.TileContext(nc) as tc:
        with (
            tc.tile_pool(name="xp", bufs=1) as xp,
            tc.tile_pool(name="hp", bufs=1) as hp,
            tc.tile_pool(name="wp", bufs=WP_BUFS) as wp,
            tc.tile_pool(name="sp", bufs=2) as sp,
            tc.tile_pool(name="yp", bufs=2) as yp,
            tc.tile_pool(name="ps", bufs=2, space=bass.MemorySpace.PSUM) as ps,
        ):
            xt_next = None
            for u_i, (x_d, wg_d, wu_d, wd_d, y_d, cap, chunks) in \
                    enumerate(units):
                # token activations, feature-major: xt[:, k, :] = X^T[k-tile]
                if xt_next is None:
                    xt = xp.tile([128, KD, cap], dt1, tag="x")
                else:
                    xt = xt_next  # prefetched during the previous phase 2
                xt_next = None

                # H^T for this unit: [f (128 part), f-tile (16), tokens]
                h = hp.tile([128, KF, cap], dt2, tag="h")

                # ---- phase 1: G/U = X @ Wg, X @ Wu ; H = silu(G) * U ----
                for fj in range(KF):
                    wgt = wp.tile([128, KD, 128], dt1, tag="wg")
                    wut = wp.tile([128, KD, 128], dt1, tag="wu")
                    if fj == 0 and u_i == 0:
                        # cold-start order: interleave the first weight tile
                        # (in halves) with x chunk 0 per k-tile so the first
                        # matmul chain is gated on the fewest possible bytes.
                        # x arrives fp16 into staging xs; the Activation
                        # engine upconverts each slice into xt right behind
                        # its DMA.
                        xs = xp.tile([128, KD, cap], FP16, tag="xs")
                        c0, w = chunks[0]
                        nc.sync.dma_start(
                            wgt[:, :KD // 2], wg_d[fj, :, :KD // 2])
                        for k in range(KD // 2):
                            nc.sync.dma_start(
                                xs[:, k, c0:c0 + w], x_d[:, k, c0:c0 + w])
                            nc.scalar.activation(
                                xt[:, k, c0:c0 + w], xs[:, k, c0:c0 + w],
                                mybir.ActivationFunctionType.Copy)
                        nc.sync.dma_start(
                            wgt[:, KD // 2:], wg_d[fj, :, KD // 2:])
                        for k in range(KD // 2, KD):
                            nc.sync.dma_start(
                                xs[:, k, c0:c0 + w], x_d[:, k, c0:c0 + w])
                            nc.scalar.activation(
                                xt[:, k, c0:c0 + w], xs[:, k, c0:c0 + w],
                                mybir.ActivationFunctionType.Copy)
                        nc.sync.dma_start(wut[:], wu_d[fj])
                        if len(chunks) > 1:
                            r0 = chunks[1][0]  # remainder of x, per k-tile
                            for k in range(KD):
                                nc.sync.dma_start(
                                    xs[:, k, r0:cap], x_d[:, k, r0:cap])
                                nc.scalar.activation(
                                    xt[:, k, r0:cap], xs[:, k, r0:cap],
                                    mybir.ActivationFunctionType.Copy)
                    else:
                        nc.sync.dma_start(wgt[:], wg_d[fj])
                        nc.sync.dma_start(wut[:], wu_d[fj])
                    for (c0, w) in chunks:
                        g_ps = ps.tile([128, w], FP32, tag="g")
                        for k in range(KD):
                            nc.tensor.matmul(
                                g_ps[:, :w],
                                wgt[:, k, :], xt[:, k, c0:c0 + w],
                                start=(k == 0), stop=(k == KD - 1),
                            )
                        u_ps = ps.tile([128, w], FP32, tag="u")
                        for k in range(KD):
                            nc.tensor.matmul(
                                u_ps[:, :w],
                                wut[:, k, :], xt[:, k, c0:c0 + w],
                                start=(k == 0), stop=(k == KD - 1),
                            )
                        sg = sp.tile([128, w], FP32, tag="sg")
                        nc.scalar.activation(
                            sg[:, :w], g_ps[:, :w],
                            mybir.ActivationFunctionType.Silu,
                        )
                        nc.vector.tensor_mul(
                            h[:, fj, c0:c0 + w], sg[:, :w], u_ps[:, :w]
                        )

                # ---- phase 2: Y^T partial = Wd^T @ H^T (accum over f) ----
                for d in range(KD):
                    wdt = wp.tile([128, KF, 128], dt2, tag="wd")
                    nc.sync.dma_start(wdt[:], wd_d[d])
                    if u_i < last_unit and d < 4:
                        # prefetch the next unit's x in quarters, interleaved
                        # with the wd stream so no single transfer
                        # head-of-line-blocks the following phase-1 weights.
                        # a-units arrive fp16 and are upconverted here, in
                        # the previous unit's phase-2 slack.
                        xn_d, cap_n = units[u_i + 1][0], units[u_i + 1][5]
                        next_a = (u_i + 1) % 2 == 0
                        if d == 0:
                            xt_next = xp.tile([128, KD, cap_n], dt1, tag="x")
                            if next_a:
                                xs_next = xp.tile(
                                    [128, KD, cap_n], FP16, tag="xs")
                        k0, k1 = 2 * d, 2 * d + 2
                        if next_a:
                            nc.sync.dma_start(
                                xs_next[:, k0:k1], xn_d[:, k0:k1])
                            nc.scalar.activation(
                                xt_next[:, k0:k1], xs_next[:, k0:k1],
                                mybir.ActivationFunctionType.Copy)
                        else:
                            nc.sync.dma_start(
                                xt_next[:, k0:k1], xn_d[:, k0:k1])
                    y_sb = yp.tile([128, cap], FP32, tag="y")
                    tail = u_i == last_unit and d == KD - 1
                    d_chunks = _tail_chunks(chunks) if tail else chunks
                    for (c0, w) in d_chunks:
                        y_ps = ps.tile([128, w], FP32, tag="yp")
                        for fj in range(KF):
                            nc.tensor.matmul(
                                y_ps[:, :w],
                                wdt[:, fj, :], h[:, fj, c0:c0 + w],
                                start=(fj == 0), stop=(fj == KF - 1),
                            )
                        nc.vector.tensor_copy(y_sb[:, c0:c0 + w], y_ps[:, :w])
                        if tail:
                            # per-chunk drain so the final flush after the
                            # last matmul is a single narrow chunk
                            nc.scalar.dma_start(
                                y_d[d][:, c0:c0 + w], y_sb[:, c0:c0 + w])
                    if not tail:
                        # one DMA per d-tile, on the Activation queue:
                        # outputs must not head-of-line-block input prefetch
                        # on the SP queue
                        nc.scalar.dma_start(y_d[d], y_sb[:])

    return nc


# ---------------------------------------------------------------------------
# Host-side sharding / unsharding
# ---------------------------------------------------------------------------

def _prep_w_gate_like(w_slice):
    """[D, F2] -> [KF, 128, KD, 128] so each f-tile is one contiguous DMA."""
    arr = w_slice.reshape(KD, 128, KF, 128)       # [k, p, fj, f]
    return np.ascontiguousarray(arr.transpose(2, 1, 0, 3)).astype(NP_DT1)


def _prep_w_down(w_slice):
    """[F2, D] -> [KD, 128, KF, 128] so each d-tile is one contiguous DMA."""
    arr = w_slice.reshape(KF, 128, KD, 128)       # [fj, p, d, dc]
    return np.ascontiguousarray(arr.transpose(2, 1, 0, 3)).astype(NP_DT2)


def _prep_x(x_tok, cap, np_dt):
    """[count, D] tokens -> [128, KD, cap] partition-major padded."""
    count = x_tok.shape[0]
    xt = np.zeros((D, cap), dtype=np_dt)
    xt[:, :count] = x_tok.T.astype(np_dt)
    return np.ascontiguousarray(xt.reshape(KD, 128, cap).transpose(1, 0, 2))


_prog_cache = {}


def get_program(caps) -> bass.Bass:
    if caps not in _prog_cache:
        _prog_cache[caps] = build_program(caps)
    return _prog_cache[caps]


def _pad16(n):
    return max(128, int(math.ceil(max(int(n), 1) / 16)) * 16)


def prepare_in_maps(np_inputs):
    """Host-side sharding.  Returns (in_maps, caps, meta) where meta carries
    what's needed to unshard.

    Experts are sorted by token count: the 4 largest go to unit a (cap0),
    the 4 smallest to unit b (cap1).  This minimizes cap0+cap1 (the per-core
    PE time is proportional to it) and, for balanced routing, gives unit b
    cap1 <= 1024 = 2x512-wide PSUM chunks."""
    x = np.asarray(np_inputs["x"])
    B, S, _ = x.shape
    xf = np.ascontiguousarray(x.reshape(-1, D).astype(np.float32, copy=False))
    idx = np.asarray(np_inputs["expert_idx"]).reshape(-1)
    w_gate = np.asarray(np_inputs["w_gate"], dtype=np.float32)
    w_up = np.asarray(np_inputs["w_up"], dtype=np.float32)
    w_down = np.asarray(np_inputs["w_down"], dtype=np.float32)

    order = np.argsort(idx, kind="stable")
    counts = np.bincount(idx, minlength=E).astype(np.int64)
    starts = np.zeros(E + 1, dtype=np.int64)
    np.cumsum(counts, out=starts[1:])

    by_count = np.argsort(-counts, kind="stable")
    big, small = by_count[:E // 2], by_count[E // 2:]
    cap0 = _pad16(counts[big].max())
    cap1 = _pad16(counts[small].max())
    caps = (cap0, cap1)

    tok_of = [order[starts[e]:starts[e + 1]] for e in range(E)]

    in_maps = []
    for pair in range(4):
        ea, eb = int(big[pair]), int(small[pair])
        xa = _prep_x(xf[tok_of[ea]], cap0, np.float16)
        xb = _prep_x(xf[tok_of[eb]], cap1, NP_DT1)
        for hhalf in range(2):
            sl = slice(hhalf * F2, (hhalf + 1) * F2)
            in_maps.append({
                "xa": xa,
                "xb": xb,
                "wga": _prep_w_gate_like(w_gate[ea][:, sl]),
                "wua": _prep_w_gate_like(w_up[ea][:, sl]),
                "wda": _prep_w_down(w_down[ea][sl, :]),
                "wgb": _prep_w_gate_like(w_gate[eb][:, sl]),
                "wub": _prep_w_gate_like(w_up[eb][:, sl]),
                "wdb": _prep_w_down(w_down[eb][sl, :]),
            })
    meta = (tok_of, counts, (B, S), x.dtype, big, small)
    return in_maps, caps, meta


def unshard(results, caps, meta):
    tok_of, counts, (B, S), out_dtype, big, small = meta
    cap0, cap1 = caps
    out = np.zeros((B * S, D), dtype=np.float32)
    for pair in range(4):
        r0 = results[2 * pair]
        r1 = results[2 * pair + 1]
        for e, key, cap in ((int(big[pair]), "ya", cap0),
                            (int(small[pair]), "yb", cap1)):
            yt = (np.asarray(r0[key], dtype=np.float32)
                  + np.asarray(r1[key], dtype=np.float32)).reshape(D, cap)
            out[tok_of[e]] = yt[:, :counts[e]].T
    return out.reshape(B, S, D).astype(out_dtype, copy=False)


def kernel(x, expert_idx, w_gate, w_up, w_down):
    np_inputs = {"x": x, "expert_idx": expert_idx, "w_gate": w_gate,
                 "w_up": w_up, "w_down": w_down}
    in_maps, caps, meta = prepare_in_maps(np_inputs)
    nc = get_program(caps)
    try:
        res = run_bass_kernel_spmd(nc, in_maps, list(range(8)))
    except Exception:
        # transient device errors have been observed on this fabric; one retry
        res = run_bass_kernel_spmd(nc, in_maps, list(range(8)))
    return unshard(res.results, caps, meta)

